# revision 12
# baseline (speedup 1.0000x reference)
"""Top-1 MoE FFN (B=1, T=2048, C=1024, F=4096, E=8) on 8 trn2 NeuronCores.

Expert parallelism: core e owns expert e's weights. The router (tiny:
T x C x E matmul + softmax + argmax) runs on host, which also performs the
dispatch (gather tokens by top-1 expert id, the all-to-all equivalent) and
the combine (scatter + top1_prob scaling). Each core runs the dense FFN
  y = silu(x_e @ w1[e] + b1[e]) @ w2[e] + b2[e]
over its (padded) token batch.

Device GEMMs run in fp32r (e8m11): full PE rate for moving free-dim >= 256,
~1.5e-4 relative error from input rounding (PSUM accumulation stays fp32).
The e8m11 round-to-nearest-even is done ON HOST (bit-exact with the chip's
DVE conversion), so the device does zero rounding work: fp32r tiles are
DMA'd directly. fp32r-typed DMA descriptors crash the HWDGE (sync) queue
but work on the SWDGE (gpsimd) queue, so all matmul-operand loads go
through nc.gpsimd.dma_start.

GEMM1 produces hT [F, tok] directly (weights stationary), which is exactly
the stationary layout GEMM2 needs, so no transposes anywhere on device
(x arrives host-transposed as xT [C, tok]).
"""
import numpy as np

import concourse.bass as bass
import concourse.mybir as mybir
import concourse.tile as tile
from concourse import bacc
from concourse.bass_utils import run_bass_kernel_spmd

B, T, C, F, E = 1, 2048, 1024, 4096, 8
N_CORES = 8

F32 = mybir.dt.float32
F32R = mybir.dt.float32r
KC = C // 128   # 8  k-tiles over C
KF = F // 128   # 32 k-tiles over F
W1G = 512       # w1 F-columns per DMA group (4 f-tiles, 2MB)
WG = 4          # w2 k-tiles per DMA group (2MB)
ACT = mybir.ActivationFunctionType


def build_program(n_pad: int):
    """Per-core FFN program: y[n_pad, C] = silu(xT.T @ w1 + b1) @ w2 + b2."""
    assert n_pad % 64 == 0 and n_pad >= 256
    mt = -(-n_pad // 128)  # token m-tiles (last may be 64 wide)

    def mw(m):  # width of token m-tile
        return min(128, n_pad - m * 128)

    nc = bacc.Bacc()
    xt = nc.declare_dram_parameter("xt", [C, n_pad], F32R, isOutput=False)
    w1 = nc.declare_dram_parameter("w1", [C, F], F32R, isOutput=False)
    w2 = nc.declare_dram_parameter("w2", [F, C], F32R, isOutput=False)
    b1t = nc.declare_dram_parameter("b1t", [128, KF], F32, isOutput=False)
    b2r = nc.declare_dram_parameter("b2r", [128, C], F32, isOutput=False)
    y = nc.declare_dram_parameter("y", [n_pad, C], F32, isOutput=True)

    # batched-DMA views: k-subtiles side by side per transfer
    xt_v = xt.rearrange("(k p) t -> p k t", p=128)            # [128, KC, n_pad]
    w2_v = w2.rearrange("(g k p) c -> g p k c", p=128, k=WG)  # [KF/WG][128, WG, C]

    with tile.TileContext(nc) as tc:
        with (
            tc.tile_pool(name="const", bufs=1) as cpool,
            tc.tile_pool(name="xr", bufs=1) as xrpool,
            tc.tile_pool(name="w1p", bufs=3) as w1pool,
            tc.tile_pool(name="h", bufs=1) as hpool,
            tc.tile_pool(name="w2p", bufs=3) as w2pool,
            tc.tile_pool(name="y", bufs=1) as ypool,
            tc.tile_pool(name="ps1", bufs=2, space="PSUM") as ps1pool,
            tc.tile_pool(name="ps2", bufs=1, space="PSUM") as ps2pool,
        ):
            b1t_sb = cpool.tile([128, KF], F32, name="b1t_sb")
            nc.sync.dma_start(out=b1t_sb[:], in_=b1t[:])
            b2r_sb = cpool.tile([128, C], F32, name="b2r_sb")
            nc.sync.dma_start(out=b2r_sb[:], in_=b2r[:])

            # xT [C, n_pad]: one fp32r DMA (fp32r works only on the gpsimd
            # SWDGE queue; HWDGE crashes on the dtype)
            xr = xrpool.tile([128, KC, n_pad], F32R, name="xr")
            nc.gpsimd.dma_start(out=xr[:], in_=xt_v)

            # GEMM1: hT[ft] [128, n_pad] = silu(sum_k w1[k,ft].T @ xT[k] + b1)
            # first groups are small so the PE starts early
            widths, f0 = [], 0
            while f0 < F:
                w = 256 if len(widths) < 2 else W1G
                widths.append((f0, min(w, F - f0)))
                f0 += widths[-1][1]
            hT = []
            for fg, (fstart, fwidth) in enumerate(widths):
                w1g = w1pool.tile([128, KC, fwidth], F32R, name=f"w1g_{fg}",
                                  tag="w1g")
                nc.gpsimd.dma_start(
                    out=w1g[:],
                    in_=w1[:, fstart:fstart + fwidth].rearrange(
                        "(k p) f -> p k f", p=128
                    ),
                )
                for fi in range(fwidth // 128):
                    ft = (fstart // 128) + fi
                    ps = ps1pool.tile([128, n_pad], F32, name=f"ps1_{ft}", tag="ps1")
                    for k in range(KC):
                        nc.tensor.matmul(
                            ps[:],
                            lhsT=w1g[:, k, fi * 128:(fi + 1) * 128],
                            rhs=xr[:, k],
                            start=(k == 0),
                            stop=(k == KC - 1),
                        )
                    h_t = hpool.tile([128, n_pad], F32R, name=f"hT_{ft}", tag=f"h{ft}")
                    nc.scalar.activation(
                        h_t[:], ps[:], ACT.Silu,
                        bias=b1t_sb[:, ft:ft + 1], scale=1.0,
                    )
                    hT.append(h_t)

            # GEMM2: y[m] [128, C] = sum_kf hT[kf][:, m].T @ w2[kf] + b2
            # token m-tiles accumulate concurrently in groups of <=3 (2 banks
            # each; ps1 keeps 2 of the 8 banks)
            for m0 in range(0, mt, 3):
                ms = range(m0, min(m0 + 3, mt))
                ps_y = {
                    m: ps2pool.tile([128, C], F32, name=f"psy_{m}", tag=f"psy{m % 3}")
                    for m in ms
                }
                for g in range(KF // WG):
                    w2g = w2pool.tile([128, WG, C], F32R, name=f"w2g_{m0}_{g}",
                                      tag="w2g")
                    nc.gpsimd.dma_start(out=w2g[:], in_=w2_v[g])
                    for j in range(WG):
                        kf = g * WG + j
                        for m in ms:
                            for ci in range(C // 512):
                                nc.tensor.matmul(
                                    ps_y[m][: mw(m), ci * 512:(ci + 1) * 512],
                                    lhsT=hT[kf][:, m * 128:m * 128 + mw(m)],
                                    rhs=w2g[:, j, ci * 512:(ci + 1) * 512],
                                    start=(kf == 0),
                                    stop=(kf == KF - 1),
                                )
                for m in ms:
                    y_sb = ypool.tile([128, C], F32, name=f"y_{m}", tag=f"y{m % 3}")
                    nc.vector.tensor_add(
                        y_sb[: mw(m)], ps_y[m][: mw(m)], b2r_sb[: mw(m)]
                    )
                    nc.sync.dma_start(
                        out=y[m * 128:m * 128 + mw(m), :], in_=y_sb[: mw(m)]
                    )
    nc.finalize()
    return nc


def _round_e8m11(v):
    """Bit-exact replica of the chip's fp32 -> fp32r conversion
    (round-to-nearest-even to 11 mantissa bits, low 12 bits zero)."""
    b = np.ascontiguousarray(v, dtype=np.float32).view(np.uint32).astype(np.uint64)
    lsb = (b >> np.uint64(12)) & np.uint64(1)
    r = (b + np.uint64(0x7FF) + lsb) & np.uint64(0xFFFFF000)
    return r.astype(np.uint32).view(np.float32)


def _router_host(x2d, router_w, router_b):
    """Softmax-order-identical router: probs, top1, p(top1)."""
    logits = x2d @ router_w + router_b          # [T, E] f32
    m = logits.max(axis=-1, keepdims=True)
    ex = np.exp(logits - m)
    probs = ex / ex.sum(axis=-1, keepdims=True)  # [T, E] f32
    top1 = np.argmax(probs, axis=-1)
    top1_prob = probs.max(axis=-1)
    return probs, top1, top1_prob


def kernel(x, router_w, router_b, w1, b1, w2, b2, _trace=False):
    x = np.asarray(x, dtype=np.float32)
    router_w = np.asarray(router_w, dtype=np.float32)
    router_b = np.asarray(router_b, dtype=np.float32)
    w1 = np.asarray(w1, dtype=np.float32)
    b1 = np.asarray(b1, dtype=np.float32)
    w2 = np.asarray(w2, dtype=np.float32)
    b2 = np.asarray(b2, dtype=np.float32)

    x2d = x.reshape(B * T, C)
    probs, top1, top1_prob = _router_host(x2d, router_w, router_b)

    # aux loss (host): E * sum(importance * load)
    importance = probs.mean(axis=0)                      # [E]
    load = (np.bincount(top1, minlength=E) / np.float32(B * T)).astype(np.float32)
    aux_loss = np.float32(E) * np.sum(importance * load, dtype=np.float32)

    # dispatch: gather tokens by expert, pad, transpose, pre-round to fp32r
    idx = [np.nonzero(top1 == e)[0] for e in range(E)]
    max_count = max(int(i.size) for i in idx)
    n_pad = max(256, -(-max_count // 64) * 64)

    in_maps = []
    for e in range(E):
        xe = np.zeros((n_pad, C), dtype=np.float32)
        xe[: idx[e].size] = x2d[idx[e]]
        in_maps.append({
            "xt": _round_e8m11(xe.T),
            "w1": _round_e8m11(w1[e]),
            "w2": _round_e8m11(w2[e]),
            "b1t": np.ascontiguousarray(b1[e].reshape(KF, 128).T),
            "b2r": np.ascontiguousarray(np.broadcast_to(b2[e], (128, C))),
        })

    nc = build_program(n_pad)
    res = run_bass_kernel_spmd(nc, in_maps, list(range(N_CORES)), trace=_trace)

    # combine: scatter + top1_prob scaling
    out2d = np.empty((B * T, C), dtype=np.float32)
    for e in range(E):
        ye = res.results[e]["y"][: idx[e].size]
        out2d[idx[e]] = ye * top1_prob[idx[e], None]
    outputs = out2d.reshape(B, T, C)

    if _trace:
        kernel.last_results = res
    return outputs, aux_loss


# revision 19
# speedup vs baseline: 1.0736x; 1.0736x over previous
"""Top-1 MoE FFN (B=1, T=2048, C=1024, F=4096, E=8) on 8 trn2 NeuronCores.

Expert parallelism: core e owns expert e's weights. The router (tiny:
T x C x E matmul + softmax + argmax) runs on host, which also performs the
dispatch (gather tokens by top-1 expert id, the all-to-all equivalent) and
the combine (scatter + top1_prob scaling). Each core runs the dense FFN
  y = silu(x_e @ w1[e] + b1[e]) @ w2[e] + b2[e]
over its (padded) token batch.

Device GEMMs run in fp32r (e8m11): full PE rate for moving free-dim >= 256,
~1.5e-4 relative error from input rounding (PSUM accumulation stays fp32).
The e8m11 round-to-nearest-even is done ON HOST (bit-exact with the chip's
DVE conversion), so the device does zero rounding work: fp32r tiles are
DMA'd directly. fp32r-typed DMA descriptors crash the HWDGE (sync) queue
but work on the SWDGE (gpsimd) queue, so all matmul-operand loads go
through nc.gpsimd.dma_start.

GEMM1 produces hT [F, tok] directly (weights stationary), which is exactly
the stationary layout GEMM2 needs, so no transposes anywhere on device
(x arrives host-transposed as xT [C, tok]).
"""
import numpy as np

import concourse.bass as bass
import concourse.mybir as mybir
import concourse.tile as tile
from concourse import bacc
from concourse.bass_utils import run_bass_kernel_spmd

B, T, C, F, E = 1, 2048, 1024, 4096, 8
N_CORES = 8

F32 = mybir.dt.float32
F32R = mybir.dt.float32r
KC = C // 128   # 8  k-tiles over C
KF = F // 128   # 32 k-tiles over F
W1G = 512       # w1 F-columns per DMA group (4 f-tiles, 2MB)
WG = 4          # w2 k-tiles per DMA group (2MB)
ACT = mybir.ActivationFunctionType


def build_program(n_pad: int):
    """Per-core FFN program: y[n_pad, C] = silu(xT.T @ w1 + b1) @ w2 + b2."""
    assert n_pad % 64 == 0 and n_pad >= 256
    mt = -(-n_pad // 128)  # token m-tiles (last may be 64 wide)

    def mw(m):  # width of token m-tile
        return min(128, n_pad - m * 128)

    nc = bacc.Bacc()
    xt = nc.declare_dram_parameter("xt", [C, n_pad], F32R, isOutput=False)
    w1 = nc.declare_dram_parameter("w1", [C, F], F32R, isOutput=False)
    w2 = nc.declare_dram_parameter("w2", [F, C], F32R, isOutput=False)
    b1t = nc.declare_dram_parameter("b1t", [128, KF], F32, isOutput=False)
    b2 = nc.declare_dram_parameter("b2", [128, C], F32, isOutput=False)
    y = nc.declare_dram_parameter("y", [n_pad, C], F32, isOutput=True)

    # batched-DMA view: k-subtiles side by side per transfer
    xt_v = xt.rearrange("(k p) t -> p k t", p=128)            # [128, KC, n_pad]

    with tile.TileContext(nc) as tc:
        with (
            tc.tile_pool(name="const", bufs=1) as cpool,
            tc.tile_pool(name="xr", bufs=1) as xrpool,
            tc.tile_pool(name="w1p", bufs=3) as w1pool,
            tc.tile_pool(name="h", bufs=1) as hpool,
            tc.tile_pool(name="w2p", bufs=3) as w2pool,
            tc.tile_pool(name="y", bufs=1) as ypool,
            tc.tile_pool(name="ps1", bufs=2, space="PSUM") as ps1pool,
            tc.tile_pool(name="ps2", bufs=1, space="PSUM") as ps2pool,
        ):
            # biases on the scalar HWDGE queue, keeping gpsimd pure weights
            b1t_sb = cpool.tile([128, KF], F32, name="b1t_sb")
            nc.scalar.dma_start(out=b1t_sb[:], in_=b1t[:])
            b2_sb = cpool.tile([128, C], F32, name="b2_sb")
            nc.scalar.dma_start(out=b2_sb[:], in_=b2[:])

            # xT [C, n_pad]: one fp32r DMA (fp32r works only on the gpsimd
            # SWDGE queue; HWDGE crashes on the dtype)
            xr = xrpool.tile([128, KC, n_pad], F32R, name="xr")
            nc.gpsimd.dma_start(out=xr[:], in_=xt_v)

            # GEMM1: hT[ft] [128, n_pad] = silu(sum_k w1[k,ft].T @ xT[k] + b1)
            # first group small so the PE starts early
            widths, f0 = [], 0
            while f0 < F:
                w = 256 if not widths else W1G
                widths.append((f0, min(w, F - f0)))
                f0 += widths[-1][1]
            hT = []
            for fg, (fstart, fwidth) in enumerate(widths):
                w1g = w1pool.tile([128, KC, fwidth], F32R, name=f"w1g_{fg}",
                                  tag="w1g")
                nc.gpsimd.dma_start(
                    out=w1g[:],
                    in_=w1[:, fstart:fstart + fwidth].rearrange(
                        "(k p) f -> p k f", p=128
                    ),
                )
                for fi in range(fwidth // 128):
                    ft = (fstart // 128) + fi
                    ps = ps1pool.tile([128, n_pad], F32, name=f"ps1_{ft}", tag="ps1")
                    for k in range(KC):
                        nc.tensor.matmul(
                            ps[:],
                            lhsT=w1g[:, k, fi * 128:(fi + 1) * 128],
                            rhs=xr[:, k],
                            start=(k == 0),
                            stop=(k == KC - 1),
                        )
                    h_t = hpool.tile([128, n_pad], F32R, name=f"hT_{ft}", tag=f"h{ft}")
                    nc.scalar.activation(
                        h_t[:], ps[:], ACT.Silu,
                        bias=b1t_sb[:, ft:ft + 1], scale=1.0,
                    )
                    hT.append(h_t)

            # GEMM2: y[m] [128, C] = sum_kf hT[kf][:, m].T @ w2[kf] + b2
            # token m-tiles accumulate concurrently in groups of <=3 (2 banks
            # each; ps1 keeps 2 of the 8 banks)
            for m0 in range(0, mt, 3):
                ms = range(m0, min(m0 + 3, mt))
                ps_y = {
                    m: ps2pool.tile([128, C], F32, name=f"psy_{m}", tag=f"psy{m % 3}")
                    for m in ms
                }
                # w2 k-groups: big in the middle, small at the end so the
                # post-last-DMA tail is short
                w2widths, k0 = [], 0
                while k0 < KF - 4:
                    w2widths.append((k0, WG))
                    k0 += WG
                w2widths += [(k0, 2), (k0 + 2, 1), (k0 + 3, 1)]
                for g, (kstart, kwidth) in enumerate(w2widths):
                    w2g = w2pool.tile([128, kwidth, C], F32R,
                                      name=f"w2g_{m0}_{g}", tag="w2g")
                    nc.gpsimd.dma_start(
                        out=w2g[:],
                        in_=w2[kstart * 128:(kstart + kwidth) * 128, :].rearrange(
                            "(k p) c -> p k c", p=128
                        ),
                    )
                    for j in range(kwidth):
                        kf = kstart + j
                        for m in ms:
                            for ci in range(C // 512):
                                nc.tensor.matmul(
                                    ps_y[m][: mw(m), ci * 512:(ci + 1) * 512],
                                    lhsT=hT[kf][:, m * 128:m * 128 + mw(m)],
                                    rhs=w2g[:, j, ci * 512:(ci + 1) * 512],
                                    start=(kf == 0),
                                    stop=(kf == KF - 1),
                                )
                for m in ms:
                    y_sb = ypool.tile([128, C], F32, name=f"y_{m}", tag=f"y{m % 3}")
                    nc.vector.tensor_add(
                        y_sb[: mw(m)], ps_y[m][: mw(m)], b2_sb[: mw(m)]
                    )
                    nc.sync.dma_start(
                        out=y[m * 128:m * 128 + mw(m), :], in_=y_sb[: mw(m)]
                    )
    nc.finalize()
    return nc


def _round_e8m11(v):
    """Bit-exact replica of the chip's fp32 -> fp32r conversion
    (round-to-nearest-even to 11 mantissa bits, low 12 bits zero)."""
    b = np.ascontiguousarray(v, dtype=np.float32).view(np.uint32).astype(np.uint64)
    lsb = (b >> np.uint64(12)) & np.uint64(1)
    r = (b + np.uint64(0x7FF) + lsb) & np.uint64(0xFFFFF000)
    return r.astype(np.uint32).view(np.float32)


def _router_host(x2d, router_w, router_b):
    """Softmax-order-identical router: probs, top1, p(top1)."""
    logits = x2d @ router_w + router_b          # [T, E] f32
    m = logits.max(axis=-1, keepdims=True)
    ex = np.exp(logits - m)
    probs = ex / ex.sum(axis=-1, keepdims=True)  # [T, E] f32
    top1 = np.argmax(probs, axis=-1)
    top1_prob = probs.max(axis=-1)
    return probs, top1, top1_prob


def kernel(x, router_w, router_b, w1, b1, w2, b2, _trace=False):
    x = np.asarray(x, dtype=np.float32)
    router_w = np.asarray(router_w, dtype=np.float32)
    router_b = np.asarray(router_b, dtype=np.float32)
    w1 = np.asarray(w1, dtype=np.float32)
    b1 = np.asarray(b1, dtype=np.float32)
    w2 = np.asarray(w2, dtype=np.float32)
    b2 = np.asarray(b2, dtype=np.float32)

    x2d = x.reshape(B * T, C)
    probs, top1, top1_prob = _router_host(x2d, router_w, router_b)

    # aux loss (host): E * sum(importance * load)
    importance = probs.mean(axis=0)                      # [E]
    load = (np.bincount(top1, minlength=E) / np.float32(B * T)).astype(np.float32)
    aux_loss = np.float32(E) * np.sum(importance * load, dtype=np.float32)

    # dispatch: gather tokens by expert, pad, transpose, pre-round to fp32r
    idx = [np.nonzero(top1 == e)[0] for e in range(E)]
    max_count = max(int(i.size) for i in idx)
    n_pad = max(256, -(-max_count // 64) * 64)

    in_maps = []
    for e in range(E):
        xe = np.zeros((n_pad, C), dtype=np.float32)
        xe[: idx[e].size] = x2d[idx[e]]
        in_maps.append({
            "xt": _round_e8m11(xe.T),
            "w1": _round_e8m11(w1[e]),
            "w2": _round_e8m11(w2[e]),
            "b1t": np.ascontiguousarray(b1[e].reshape(KF, 128).T),
            "b2": np.ascontiguousarray(np.broadcast_to(b2[e], (128, C))),
        })

    nc = build_program(n_pad)
    res = run_bass_kernel_spmd(nc, in_maps, list(range(N_CORES)), trace=_trace)

    # combine: scatter + top1_prob scaling
    out2d = np.empty((B * T, C), dtype=np.float32)
    for e in range(E):
        ye = res.results[e]["y"][: idx[e].size]
        out2d[idx[e]] = ye * top1_prob[idx[e], None]
    outputs = out2d.reshape(B, T, C)

    if _trace:
        kernel.last_results = res
    return outputs, aux_loss


# revision 20
# speedup vs baseline: 1.2193x; 1.1357x over previous
"""Top-1 MoE FFN (B=1, T=2048, C=1024, F=4096, E=8) on 8 trn2 NeuronCores.

Expert parallelism: core e owns expert e's weights. The router (tiny:
T x C x E matmul + softmax + argmax) runs on host, which also performs the
dispatch (gather tokens by top-1 expert id, the all-to-all equivalent) and
the combine (scatter + top1_prob scaling). Each core runs the dense FFN
  y = silu(x_e @ w1[e] + b1[e]) @ w2[e] + b2[e]
over its (padded) token batch.

Device GEMMs run in fp32r (e8m11): full PE rate for moving free-dim >= 256,
~1.5e-4 relative error from input rounding (PSUM accumulation stays fp32).
The e8m11 round-to-nearest-even is done ON HOST (bit-exact with the chip's
DVE conversion), so the device does zero rounding work: fp32r tiles are
DMA'd directly. fp32r-typed DMA descriptors crash the HWDGE (sync) queue
but work on the SWDGE (gpsimd) queue, so all matmul-operand loads go
through nc.gpsimd.dma_start.

GEMM1 produces hT [F, tok] directly (weights stationary), which is exactly
the stationary layout GEMM2 needs, so no transposes anywhere on device
(x arrives host-transposed as xT [C, tok]).
"""
import numpy as np

import concourse.bass as bass
import concourse.mybir as mybir
import concourse.tile as tile
from concourse import bacc
from concourse.bass_utils import run_bass_kernel_spmd

B, T, C, F, E = 1, 2048, 1024, 4096, 8
N_CORES = 8

F32 = mybir.dt.float32
F32R = mybir.dt.float32r
KC = C // 128   # 8  k-tiles over C
KF = F // 128   # 32 k-tiles over F
W1G = 512       # w1 F-columns per DMA group (4 f-tiles, 2MB)
WG = 4          # w2 k-tiles per DMA group (2MB)
ACT = mybir.ActivationFunctionType


def build_program(n_pad: int):
    """Per-core FFN program: y[n_pad, C] = silu(xT.T @ w1 + b1) @ w2 + b2."""
    assert n_pad % 64 == 0 and n_pad >= 256
    mt = -(-n_pad // 128)  # token m-tiles (last may be 64 wide)

    def mw(m):  # width of token m-tile
        return min(128, n_pad - m * 128)

    nc = bacc.Bacc()
    # all matmul operands arrive host-pre-tiled: each DMA group is one
    # fully-contiguous [128, bytes] block (max DMA line efficiency)
    xt = nc.declare_dram_parameter("xt", [128, KC * n_pad], F32R, isOutput=False)
    w1 = nc.declare_dram_parameter("w1", [128, KC * F], F32R, isOutput=False)
    w2 = nc.declare_dram_parameter("w2", [128, KF * C], F32R, isOutput=False)
    b1t = nc.declare_dram_parameter("b1t", [128, KF], F32, isOutput=False)
    b2 = nc.declare_dram_parameter("b2", [128, C], F32, isOutput=False)
    y = nc.declare_dram_parameter("y", [n_pad, C], F32, isOutput=True)

    with tile.TileContext(nc) as tc:
        with (
            tc.tile_pool(name="const", bufs=1) as cpool,
            tc.tile_pool(name="xr", bufs=1) as xrpool,
            tc.tile_pool(name="w1p", bufs=3) as w1pool,
            tc.tile_pool(name="h", bufs=1) as hpool,
            tc.tile_pool(name="w2p", bufs=3) as w2pool,
            tc.tile_pool(name="y", bufs=1) as ypool,
            tc.tile_pool(name="ps1", bufs=2, space="PSUM") as ps1pool,
            tc.tile_pool(name="ps2", bufs=1, space="PSUM") as ps2pool,
        ):
            # biases on the scalar HWDGE queue, keeping gpsimd pure weights
            b1t_sb = cpool.tile([128, KF], F32, name="b1t_sb")
            nc.scalar.dma_start(out=b1t_sb[:], in_=b1t[:])
            b2_sb = cpool.tile([128, C], F32, name="b2_sb")
            nc.scalar.dma_start(out=b2_sb[:], in_=b2[:])

            # xT [C, n_pad]: one fp32r DMA (fp32r works only on the gpsimd
            # SWDGE queue; HWDGE crashes on the dtype)
            xr = xrpool.tile([128, KC, n_pad], F32R, name="xr")
            nc.gpsimd.dma_start(
                out=xr[:], in_=xt[:].rearrange("p (k t) -> p k t", k=KC)
            )

            # GEMM1: hT[ft] [128, n_pad] = silu(sum_k w1[k,ft].T @ xT[k] + b1)
            hT = []
            for fg, (fstart, fwidth) in enumerate(w1_groups()):
                w1g = w1pool.tile([128, KC, fwidth], F32R, name=f"w1g_{fg}",
                                  tag="w1g")
                nc.gpsimd.dma_start(
                    out=w1g[:],
                    in_=w1[:, fstart * KC:(fstart + fwidth) * KC].rearrange(
                        "p (k f) -> p k f", k=KC
                    ),
                )
                for fi in range(fwidth // 128):
                    ft = (fstart // 128) + fi
                    ps = ps1pool.tile([128, n_pad], F32, name=f"ps1_{ft}", tag="ps1")
                    for k in range(KC):
                        nc.tensor.matmul(
                            ps[:],
                            lhsT=w1g[:, k, fi * 128:(fi + 1) * 128],
                            rhs=xr[:, k],
                            start=(k == 0),
                            stop=(k == KC - 1),
                        )
                    h_t = hpool.tile([128, n_pad], F32R, name=f"hT_{ft}", tag=f"h{ft}")
                    nc.scalar.activation(
                        h_t[:], ps[:], ACT.Silu,
                        bias=b1t_sb[:, ft:ft + 1], scale=1.0,
                    )
                    hT.append(h_t)

            # GEMM2: y[m] [128, C] = sum_kf hT[kf][:, m].T @ w2[kf] + b2
            # token m-tiles accumulate concurrently in groups of <=3 (2 banks
            # each; ps1 keeps 2 of the 8 banks)
            for m0 in range(0, mt, 3):
                ms = range(m0, min(m0 + 3, mt))
                ps_y = {
                    m: ps2pool.tile([128, C], F32, name=f"psy_{m}", tag=f"psy{m % 3}")
                    for m in ms
                }
                for g, (kstart, kwidth) in enumerate(w2_groups()):
                    w2g = w2pool.tile([128, kwidth, C], F32R,
                                      name=f"w2g_{m0}_{g}", tag="w2g")
                    nc.gpsimd.dma_start(
                        out=w2g[:],
                        in_=w2[:, kstart * C:(kstart + kwidth) * C].rearrange(
                            "p (k c) -> p k c", k=kwidth
                        ),
                    )
                    for j in range(kwidth):
                        kf = kstart + j
                        for m in ms:
                            for ci in range(C // 512):
                                nc.tensor.matmul(
                                    ps_y[m][: mw(m), ci * 512:(ci + 1) * 512],
                                    lhsT=hT[kf][:, m * 128:m * 128 + mw(m)],
                                    rhs=w2g[:, j, ci * 512:(ci + 1) * 512],
                                    start=(kf == 0),
                                    stop=(kf == KF - 1),
                                )
                for m in ms:
                    y_sb = ypool.tile([128, C], F32, name=f"y_{m}", tag=f"y{m % 3}")
                    nc.vector.tensor_add(
                        y_sb[: mw(m)], ps_y[m][: mw(m)], b2_sb[: mw(m)]
                    )
                    nc.sync.dma_start(
                        out=y[m * 128:m * 128 + mw(m), :], in_=y_sb[: mw(m)]
                    )
    nc.finalize()
    return nc


def w1_groups():
    """(fstart, fwidth) F-column groups for the w1 stream."""
    return [(f0, W1G) for f0 in range(0, F, W1G)]


def w2_groups():
    """(kstart, kwidth) F-k-tile groups for the w2 stream; small tail groups
    keep the post-last-DMA critical path short."""
    g, k0 = [], 0
    while k0 < KF - 4:
        g.append((k0, WG))
        k0 += WG
    g += [(k0, 2), (k0 + 2, 1), (k0 + 3, 1)]
    return g


def _pack_ktiles(a2d, groups, width_scale):
    """[K*128, X] -> [128, K*X] laid out so each group is one contiguous
    [128, kwidth*gwidth] block (k-subtiles side by side within a group)."""
    K128, X = a2d.shape
    blocks = []
    for start, width in groups:
        if width_scale == "cols":   # group = column range [start, start+width)
            blk = a2d[:, start:start + width]          # [K*128, width]
            blk = blk.reshape(-1, 128, width)          # [K, 128, width]
        else:                       # group = k-tile row range
            blk = a2d[start * 128:(start + width) * 128, :]
            blk = blk.reshape(width, 128, X)           # [kw, 128, X]
        blocks.append(blk.transpose(1, 0, 2).reshape(128, -1))
    return np.ascontiguousarray(np.concatenate(blocks, axis=1))


def _round_e8m11(v):
    """Bit-exact replica of the chip's fp32 -> fp32r conversion
    (round-to-nearest-even to 11 mantissa bits, low 12 bits zero)."""
    b = np.ascontiguousarray(v, dtype=np.float32).view(np.uint32).astype(np.uint64)
    lsb = (b >> np.uint64(12)) & np.uint64(1)
    r = (b + np.uint64(0x7FF) + lsb) & np.uint64(0xFFFFF000)
    return r.astype(np.uint32).view(np.float32)


def _router_host(x2d, router_w, router_b):
    """Softmax-order-identical router: probs, top1, p(top1)."""
    logits = x2d @ router_w + router_b          # [T, E] f32
    m = logits.max(axis=-1, keepdims=True)
    ex = np.exp(logits - m)
    probs = ex / ex.sum(axis=-1, keepdims=True)  # [T, E] f32
    top1 = np.argmax(probs, axis=-1)
    top1_prob = probs.max(axis=-1)
    return probs, top1, top1_prob


def kernel(x, router_w, router_b, w1, b1, w2, b2, _trace=False):
    x = np.asarray(x, dtype=np.float32)
    router_w = np.asarray(router_w, dtype=np.float32)
    router_b = np.asarray(router_b, dtype=np.float32)
    w1 = np.asarray(w1, dtype=np.float32)
    b1 = np.asarray(b1, dtype=np.float32)
    w2 = np.asarray(w2, dtype=np.float32)
    b2 = np.asarray(b2, dtype=np.float32)

    x2d = x.reshape(B * T, C)
    probs, top1, top1_prob = _router_host(x2d, router_w, router_b)

    # aux loss (host): E * sum(importance * load)
    importance = probs.mean(axis=0)                      # [E]
    load = (np.bincount(top1, minlength=E) / np.float32(B * T)).astype(np.float32)
    aux_loss = np.float32(E) * np.sum(importance * load, dtype=np.float32)

    # dispatch: gather tokens by expert, pad, transpose, pre-round to fp32r
    idx = [np.nonzero(top1 == e)[0] for e in range(E)]
    max_count = max(int(i.size) for i in idx)
    n_pad = max(256, -(-max_count // 64) * 64)

    in_maps = []
    for e in range(E):
        xe = np.zeros((n_pad, C), dtype=np.float32)
        xe[: idx[e].size] = x2d[idx[e]]
        xt_p = xe.T.reshape(KC, 128, n_pad).transpose(1, 0, 2).reshape(128, -1)
        in_maps.append({
            "xt": _round_e8m11(np.ascontiguousarray(xt_p)),
            "w1": _round_e8m11(_pack_ktiles(w1[e], w1_groups(), "cols")),
            "w2": _round_e8m11(_pack_ktiles(w2[e], w2_groups(), "rows")),
            "b1t": np.ascontiguousarray(b1[e].reshape(KF, 128).T),
            "b2": np.ascontiguousarray(np.broadcast_to(b2[e], (128, C))),
        })

    nc = build_program(n_pad)
    res = run_bass_kernel_spmd(nc, in_maps, list(range(N_CORES)), trace=_trace)

    # combine: scatter + top1_prob scaling
    out2d = np.empty((B * T, C), dtype=np.float32)
    for e in range(E):
        ye = res.results[e]["y"][: idx[e].size]
        out2d[idx[e]] = ye * top1_prob[idx[e], None]
    outputs = out2d.reshape(B, T, C)

    if _trace:
        kernel.last_results = res
    return outputs, aux_loss


# revision 21
# speedup vs baseline: 1.2635x; 1.0363x over previous
"""Top-1 MoE FFN (B=1, T=2048, C=1024, F=4096, E=8) on 8 trn2 NeuronCores.

Expert parallelism: core e owns expert e's weights. The router (tiny:
T x C x E matmul + softmax + argmax) runs on host, which also performs the
dispatch (gather tokens by top-1 expert id, the all-to-all equivalent) and
the combine (scatter + top1_prob scaling). Each core runs the dense FFN
  y = silu(x_e @ w1[e] + b1[e]) @ w2[e] + b2[e]
over its (padded) token batch.

Device GEMMs run in fp32r (e8m11): full PE rate for moving free-dim >= 256,
~1.5e-4 relative error from input rounding (PSUM accumulation stays fp32).
The e8m11 round-to-nearest-even is done ON HOST (bit-exact with the chip's
DVE conversion), so the device does zero rounding work: fp32r tiles are
DMA'd directly. fp32r-typed DMA descriptors crash the HWDGE (sync) queue
but work on the SWDGE (gpsimd) queue, so all matmul-operand loads go
through nc.gpsimd.dma_start.

GEMM1 produces hT [F, tok] directly (weights stationary), which is exactly
the stationary layout GEMM2 needs, so no transposes anywhere on device
(x arrives host-transposed as xT [C, tok]).
"""
import numpy as np

import concourse.bass as bass
import concourse.mybir as mybir
import concourse.tile as tile
from concourse import bacc
from concourse.bass_utils import run_bass_kernel_spmd

B, T, C, F, E = 1, 2048, 1024, 4096, 8
N_CORES = 8

F32 = mybir.dt.float32
F32R = mybir.dt.float32r
KC = C // 128   # 8  k-tiles over C
KF = F // 128   # 32 k-tiles over F
W1G = 512       # w1 F-columns per DMA group (4 f-tiles, 2MB)
WG = 4          # w2 k-tiles per DMA group (2MB)
ACT = mybir.ActivationFunctionType


def build_program(n_pad: int):
    """Per-core FFN program: y[n_pad, C] = silu(xT.T @ w1 + b1) @ w2 + b2."""
    assert n_pad % 64 == 0 and n_pad >= 256
    mt = -(-n_pad // 128)  # token m-tiles (last may be 64 wide)

    def mw(m):  # width of token m-tile
        return min(128, n_pad - m * 128)

    nc = bacc.Bacc()
    # all matmul operands arrive host-pre-tiled: each DMA group is one
    # fully-contiguous [128, bytes] block (max DMA line efficiency)
    xt = nc.declare_dram_parameter("xt", [128, KC * n_pad], F32, isOutput=False)
    w1 = nc.declare_dram_parameter("w1", [128, KC * F], F32R, isOutput=False)
    w2 = nc.declare_dram_parameter("w2", [128, KF * C], F32R, isOutput=False)
    b1t = nc.declare_dram_parameter("b1t", [128, KF], F32, isOutput=False)
    b2 = nc.declare_dram_parameter("b2", [128, C], F32, isOutput=False)
    y = nc.declare_dram_parameter("y", [n_pad, C], F32, isOutput=True)

    with tile.TileContext(nc) as tc:
        with (
            tc.tile_pool(name="const", bufs=1) as cpool,
            tc.tile_pool(name="xs", bufs=1) as xspool,
            tc.tile_pool(name="xr", bufs=1) as xrpool,
            tc.tile_pool(name="w1p", bufs=3) as w1pool,
            tc.tile_pool(name="h", bufs=1) as hpool,
            tc.tile_pool(name="w2p", bufs=3) as w2pool,
            tc.tile_pool(name="y", bufs=1) as ypool,
            tc.tile_pool(name="ps1", bufs=2, space="PSUM") as ps1pool,
            tc.tile_pool(name="ps2", bufs=1, space="PSUM") as ps2pool,
        ):
            # biases on the scalar HWDGE queue, keeping gpsimd pure weights
            b1t_sb = cpool.tile([128, KF], F32, name="b1t_sb")
            nc.scalar.dma_start(out=b1t_sb[:], in_=b1t[:])
            b2_sb = cpool.tile([128, C], F32, name="b2_sb")
            nc.scalar.dma_start(out=b2_sb[:], in_=b2[:])

            # xT: f32 bits (host pre-rounded) via the sync HWDGE queue so it
            # overlaps the first w1 group on gpsimd; a DVE pass-through copy
            # retypes to f32r (bits already rounded -> exact)
            xs = xspool.tile([128, KC, n_pad], F32, name="xs")
            nc.sync.dma_start(
                out=xs[:], in_=xt[:].rearrange("p (k t) -> p k t", k=KC)
            )
            xr = xrpool.tile([128, KC, n_pad], F32R, name="xr")
            nc.vector.tensor_copy(xr[:], xs[:].bitcast(F32R))

            # GEMM1: hT[ft] [128, n_pad] = silu(sum_k w1[k,ft].T @ xT[k] + b1)
            hT = []
            for fg, (fstart, fwidth) in enumerate(w1_groups()):
                w1g = w1pool.tile([128, KC, fwidth], F32R, name=f"w1g_{fg}",
                                  tag="w1g")
                nc.gpsimd.dma_start(
                    out=w1g[:],
                    in_=w1[:, fstart * KC:(fstart + fwidth) * KC].rearrange(
                        "p (k f) -> p k f", k=KC
                    ),
                )
                for fi in range(fwidth // 128):
                    ft = (fstart // 128) + fi
                    ps = ps1pool.tile([128, n_pad], F32, name=f"ps1_{ft}", tag="ps1")
                    for k in range(KC):
                        nc.tensor.matmul(
                            ps[:],
                            lhsT=w1g[:, k, fi * 128:(fi + 1) * 128],
                            rhs=xr[:, k],
                            start=(k == 0),
                            stop=(k == KC - 1),
                        )
                    h_t = hpool.tile([128, n_pad], F32R, name=f"hT_{ft}", tag=f"h{ft}")
                    nc.scalar.activation(
                        h_t[:], ps[:], ACT.Silu,
                        bias=b1t_sb[:, ft:ft + 1], scale=1.0,
                    )
                    hT.append(h_t)

            # GEMM2: y[m] [128, C] = sum_kf hT[kf][:, m].T @ w2[kf] + b2
            # token m-tiles accumulate concurrently in groups of <=3 (2 banks
            # each; ps1 keeps 2 of the 8 banks)
            for m0 in range(0, mt, 3):
                ms = range(m0, min(m0 + 3, mt))
                ps_y = {
                    m: ps2pool.tile([128, C], F32, name=f"psy_{m}", tag=f"psy{m % 3}")
                    for m in ms
                }
                for g, (kstart, kwidth) in enumerate(w2_groups()):
                    w2g = w2pool.tile([128, kwidth, C], F32R,
                                      name=f"w2g_{m0}_{g}", tag="w2g")
                    nc.gpsimd.dma_start(
                        out=w2g[:],
                        in_=w2[:, kstart * C:(kstart + kwidth) * C].rearrange(
                            "p (k c) -> p k c", k=kwidth
                        ),
                    )
                    for j in range(kwidth):
                        kf = kstart + j
                        for m in ms:
                            for ci in range(C // 512):
                                nc.tensor.matmul(
                                    ps_y[m][: mw(m), ci * 512:(ci + 1) * 512],
                                    lhsT=hT[kf][:, m * 128:m * 128 + mw(m)],
                                    rhs=w2g[:, j, ci * 512:(ci + 1) * 512],
                                    start=(kf == 0),
                                    stop=(kf == KF - 1),
                                )
                for m in ms:
                    y_sb = ypool.tile([128, C], F32, name=f"y_{m}", tag=f"y{m % 3}")
                    nc.vector.tensor_add(
                        y_sb[: mw(m)], ps_y[m][: mw(m)], b2_sb[: mw(m)]
                    )
                    nc.sync.dma_start(
                        out=y[m * 128:m * 128 + mw(m), :], in_=y_sb[: mw(m)]
                    )
    nc.finalize()
    return nc


def w1_groups():
    """(fstart, fwidth) F-column groups for the w1 stream."""
    return [(f0, W1G) for f0 in range(0, F, W1G)]


def w2_groups():
    """(kstart, kwidth) F-k-tile groups for the w2 stream; small tail groups
    keep the post-last-DMA critical path short."""
    g, k0 = [], 0
    while k0 < KF - 4:
        g.append((k0, WG))
        k0 += WG
    g += [(k0, 2), (k0 + 2, 2)]
    return g


def _pack_ktiles(a2d, groups, width_scale):
    """[K*128, X] -> [128, K*X] laid out so each group is one contiguous
    [128, kwidth*gwidth] block (k-subtiles side by side within a group)."""
    K128, X = a2d.shape
    blocks = []
    for start, width in groups:
        if width_scale == "cols":   # group = column range [start, start+width)
            blk = a2d[:, start:start + width]          # [K*128, width]
            blk = blk.reshape(-1, 128, width)          # [K, 128, width]
        else:                       # group = k-tile row range
            blk = a2d[start * 128:(start + width) * 128, :]
            blk = blk.reshape(width, 128, X)           # [kw, 128, X]
        blocks.append(blk.transpose(1, 0, 2).reshape(128, -1))
    return np.ascontiguousarray(np.concatenate(blocks, axis=1))


def _round_e8m11(v):
    """Bit-exact replica of the chip's fp32 -> fp32r conversion
    (round-to-nearest-even to 11 mantissa bits, low 12 bits zero)."""
    b = np.ascontiguousarray(v, dtype=np.float32).view(np.uint32).astype(np.uint64)
    lsb = (b >> np.uint64(12)) & np.uint64(1)
    r = (b + np.uint64(0x7FF) + lsb) & np.uint64(0xFFFFF000)
    return r.astype(np.uint32).view(np.float32)


def _router_host(x2d, router_w, router_b):
    """Softmax-order-identical router: probs, top1, p(top1)."""
    logits = x2d @ router_w + router_b          # [T, E] f32
    m = logits.max(axis=-1, keepdims=True)
    ex = np.exp(logits - m)
    probs = ex / ex.sum(axis=-1, keepdims=True)  # [T, E] f32
    top1 = np.argmax(probs, axis=-1)
    top1_prob = probs.max(axis=-1)
    return probs, top1, top1_prob


def kernel(x, router_w, router_b, w1, b1, w2, b2, _trace=False):
    x = np.asarray(x, dtype=np.float32)
    router_w = np.asarray(router_w, dtype=np.float32)
    router_b = np.asarray(router_b, dtype=np.float32)
    w1 = np.asarray(w1, dtype=np.float32)
    b1 = np.asarray(b1, dtype=np.float32)
    w2 = np.asarray(w2, dtype=np.float32)
    b2 = np.asarray(b2, dtype=np.float32)

    x2d = x.reshape(B * T, C)
    probs, top1, top1_prob = _router_host(x2d, router_w, router_b)

    # aux loss (host): E * sum(importance * load)
    importance = probs.mean(axis=0)                      # [E]
    load = (np.bincount(top1, minlength=E) / np.float32(B * T)).astype(np.float32)
    aux_loss = np.float32(E) * np.sum(importance * load, dtype=np.float32)

    # dispatch: gather tokens by expert, pad, transpose, pre-round to fp32r
    idx = [np.nonzero(top1 == e)[0] for e in range(E)]
    max_count = max(int(i.size) for i in idx)
    n_pad = max(256, -(-max_count // 64) * 64)

    in_maps = []
    for e in range(E):
        xe = np.zeros((n_pad, C), dtype=np.float32)
        xe[: idx[e].size] = x2d[idx[e]]
        xt_p = xe.T.reshape(KC, 128, n_pad).transpose(1, 0, 2).reshape(128, -1)
        in_maps.append({
            "xt": _round_e8m11(np.ascontiguousarray(xt_p)),
            "w1": _round_e8m11(_pack_ktiles(w1[e], w1_groups(), "cols")),
            "w2": _round_e8m11(_pack_ktiles(w2[e], w2_groups(), "rows")),
            "b1t": np.ascontiguousarray(b1[e].reshape(KF, 128).T),
            "b2": np.ascontiguousarray(np.broadcast_to(b2[e], (128, C))),
        })

    nc = build_program(n_pad)
    res = run_bass_kernel_spmd(nc, in_maps, list(range(N_CORES)), trace=_trace)

    # combine: scatter + top1_prob scaling
    out2d = np.empty((B * T, C), dtype=np.float32)
    for e in range(E):
        ye = res.results[e]["y"][: idx[e].size]
        out2d[idx[e]] = ye * top1_prob[idx[e], None]
    outputs = out2d.reshape(B, T, C)

    if _trace:
        kernel.last_results = res
    return outputs, aux_loss
